# revision 24
# baseline (speedup 1.0000x reference)
"""BiMamba Trainium2 kernel.

Sharding: 8 cores = (direction f/r) x (batch 2) x (d_inner half 2), SPMD
(one program, per-core data).  The host permutes channel order so each
core's own 512 scan channels occupy positions 0..511; xi/conv are computed
for all 1024 channels on every core (x_proj needs the full d_inner
contraction) with the other half's x_proj contribution accumulated into
PSUM on the fly; z/dt/scan/out_proj cover only the own half.  Partial
out_proj results are summed on the host; the reverse direction is flipped
on the host.

Device pipeline (feature-major [feature, token] layouts, f32r matmuls):
  A) in_proj -> xi; depthwise conv as 4 accumulated diag matmuls;
     silu via sigmoid*x; x_proj accumulated over all 8 channel tiles;
     z -> silu -> gT
  B) x_proj psum -> dt_raw/B/C tiles; dt_proj -> softplus(ln(1+exp)) ->
     dtT; u = dt*xc; y := D*xc
  C) per (state s, token half): broadcast B_s/C_s to 128 partitions via
     selector matmuls into PSUM; per channel tile: dA = exp(A_s*dt) on
     ScalarE, dBu = u*B_bc, tensor_tensor_scan on VectorE (carry chained
     across halves), y += h*C_bc
  D) y = y * silu(z); out_proj partial -> DRAM
"""
import os
from contextlib import ExitStack

import numpy as np

import concourse.bacc as bacc
import concourse.tile as tile
from concourse import mybir
from concourse.bass_utils import run_bass_kernel_spmd

F32 = mybir.dt.float32
BF16 = mybir.dt.bfloat16
F32R = mybir.dt.float32r
AF = mybir.ActivationFunctionType
OP = mybir.AluOpType

DIM = 512
D_STATE = 16
D_CONV = 4
D_INNER = 1024
DT_RANK = 32
B_SZ = 2
SEQ = 2048
HSEQ = SEQ // 2
HALF = 512
NPT = HALF // 128     # 4 own-channel partition tiles
NFT = D_INNER // 128  # 8 full-channel partition tiles
NC_ = SEQ // 512      # 4 token chunks
NXD = DT_RANK + 2 * D_STATE  # 64

_PROG_CACHE = {}


def _build_program():
    if "nc" in _PROG_CACHE:
        return _PROG_CACHE["nc"]
    nc = bacc.Bacc("TRN2", target_bir_lowering=False, debug=False)

    xT = nc.dram_tensor("xT", [128, 4, SEQ], F32R, kind="ExternalInput")
    w_in = nc.dram_tensor("w_in", [128, 4, D_INNER + HALF], F32R, kind="ExternalInput")
    convdiag = nc.dram_tensor("convdiag", [128, D_CONV, NFT, 128], F32R, kind="ExternalInput")
    convb = nc.dram_tensor("convb", [128, NFT, 1], F32, kind="ExternalInput")
    w_xp = nc.dram_tensor("w_xp", [128, NFT, NXD], F32R, kind="ExternalInput")
    sel = nc.dram_tensor("sel", [32, 2, D_STATE, 128], F32R, kind="ExternalInput")
    w_dt = nc.dram_tensor("w_dt", [DT_RANK, HALF], F32R, kind="ExternalInput")
    dtb = nc.dram_tensor("dtb", [128, NPT, 1], F32, kind="ExternalInput")
    Acol = nc.dram_tensor("Acol", [128, NPT, D_STATE], F32, kind="ExternalInput")
    Dv = nc.dram_tensor("Dv", [128, NPT, 1], F32, kind="ExternalInput")
    w_out = nc.dram_tensor("w_out", [128, NPT, DIM], F32R, kind="ExternalInput")
    zero3 = nc.dram_tensor("zero3", [128, 3], F32R, kind="ExternalInput")
    oT = nc.dram_tensor("oT", [128, 4, SEQ], F32, kind="ExternalOutput")

    with tile.TileContext(nc) as tc, ExitStack() as est:
        pP = est.enter_context(tc.tile_pool(name="pP", bufs=1))
        psA = est.enter_context(tc.tile_pool(name="psA", bufs=3, space="PSUM"))

        gT = pP.tile([128, NPT, SEQ], F32)        # silu(z), own half
        y_acc = pP.tile([128, NPT, SEQ], F32)
        xc_own = pP.tile([128, NPT, SEQ], F32R)   # silu(conv(xi)), own half
        dbc_raw = pP.tile([DT_RANK, SEQ], F32R)   # dt_raw
        dbcBC = pP.tile([2 * D_STATE, SEQ], F32R)  # rows 0..15 = B, 16..31 = C

        with tc.tile_pool(name="psX", bufs=4, space="PSUM") as psX:
            # x_proj accumulators: one PSUM bank per token chunk
            psx = []
            for _c in range(NC_):
                psx_t = psX.tile([NXD, 512], F32, tag="xp")
                psx.append(psx_t)

            # ---------- Phase A ----------
            with tc.tile_pool(name="pA", bufs=1) as pA, \
                 tc.tile_pool(name="pAw", bufs=2) as pAw, \
                 tc.tile_pool(name="pXi", bufs=2) as pXi:
                sb_xT = pA.tile([128, 4, SEQ], F32R)
                sb_cb = pA.tile([128, NFT, 1], F32)
                sb_wxp = pA.tile([128, NFT, NXD], F32R)
                nc.sync.dma_start(sb_xT[:], xT[:])
                nc.sync.dma_start(sb_cb[:], convb[:])
                nc.sync.dma_start(sb_wxp[:], w_xp[:])

                for m in range(12):
                    win_m = pAw.tile([128, 4, 128], F32R, tag="win")
                    nc.sync.dma_start(win_m[:], w_in[:, :, m * 128:(m + 1) * 128])
                    xi_pad = None
                    if m < 8:
                        xi_pad = pXi.tile([128, 3 + SEQ], F32R, tag="xi_pad")
                        nc.sync.dma_start(xi_pad[:, 0:3], zero3[:])
                    for c in range(NC_):
                        ps = psA.tile([128, 512], F32, tag="mm")
                        for k in range(4):
                            nc.tensor.matmul(
                                ps[:], win_m[:, k, :],
                                sb_xT[:, k, c * 512:(c + 1) * 512],
                                start=(k == 0), stop=(k == 3))
                        if m < 8:
                            nc.scalar.activation(
                                xi_pad[:, 3 + c * 512: 3 + (c + 1) * 512], ps[:], AF.Copy)
                        else:
                            # silu(z) = z * sigmoid(z)
                            sgz = pXi.tile([128, 512], F32, tag="sgz")
                            nc.scalar.activation(sgz[:], ps[:], AF.Sigmoid)
                            nc.vector.tensor_mul(
                                gT[:, m - 8, c * 512:(c + 1) * 512], ps[:], sgz[:])
                    if m < 8:
                        cd_m = pAw.tile([128, D_CONV, 128], F32R, tag="cd")
                        nc.sync.dma_start(cd_m[:], convdiag[:, :, m, :])
                        for c in range(NC_):
                            ps2 = psA.tile([128, 512], F32, tag="mm")
                            for k in range(D_CONV):
                                nc.tensor.matmul(
                                    ps2[:], cd_m[:, k, :],
                                    xi_pad[:, c * 512 + k: c * 512 + k + 512],
                                    start=(k == 0), stop=(k == D_CONV - 1))
                            if m < NPT:
                                xco = xc_own[:, m, c * 512:(c + 1) * 512]
                            else:
                                xco_t = pXi.tile([128, 512], F32R, tag="xco")
                                xco = xco_t[:]
                            # silu(v) = sigmoid(v) * v, v = conv psum + bias
                            sgc = pXi.tile([128, 512], F32, tag="sgc")
                            nc.scalar.activation(sgc[:], ps2[:], AF.Sigmoid,
                                                 bias=sb_cb[:, m, :])
                            nc.vector.scalar_tensor_tensor(
                                xco, ps2[:], sb_cb[:, m, :], sgc[:],
                                OP.add, OP.mult)
                            # accumulate x_proj contribution of this tile
                            nc.tensor.matmul(
                                psx[c][:], sb_wxp[:, m, :], xco,
                                start=(m == 0), stop=(m == 7))

            # unload x_proj accumulators (still inside psX scope)
            for c in range(NC_):
                nc.vector.tensor_copy(dbc_raw[:, c * 512:(c + 1) * 512],
                                      psx[c][0:DT_RANK, :])
                nc.vector.tensor_copy(dbcBC[:, c * 512:(c + 1) * 512],
                                      psx[c][DT_RANK:NXD, :])

        # ---------- Phase B ----------
        pBCD = est.enter_context(tc.tile_pool(name="pBCD", bufs=1))
        dtT = pBCD.tile([128, NPT, SEQ], BF16)
        uT = pBCD.tile([128, NPT, SEQ], BF16)
        sb_A = pBCD.tile([128, NPT, D_STATE], F32)
        sb_D = pBCD.tile([128, NPT, 1], F32)
        nc.sync.dma_start(sb_A[:], Acol[:])
        nc.sync.dma_start(sb_D[:], Dv[:])

        with tc.tile_pool(name="pB", bufs=1) as pB, \
             tc.tile_pool(name="pBt", bufs=2) as pBt:
            sb_wdt = pB.tile([DT_RANK, HALF], F32R)
            sb_dtb = pB.tile([128, NPT, 1], F32)
            nc.sync.dma_start(sb_wdt[:], w_dt[:])
            nc.sync.dma_start(sb_dtb[:], dtb[:])
            for mt in range(NPT):
                for c in range(NC_):
                    ps3 = psA.tile([128, 512], F32, tag="mm")
                    nc.tensor.matmul(
                        ps3[:], sb_wdt[:, mt * 128:(mt + 1) * 128],
                        dbc_raw[:, c * 512:(c + 1) * 512], start=True, stop=True)
                    # softplus(w) = ln(1 + exp(w)); w = psum + dt_bias
                    spe = pBt.tile([128, 512], F32, tag="spe")
                    nc.scalar.activation(spe[:], ps3[:], AF.Exp, bias=sb_dtb[:, mt, :])
                    nc.scalar.activation(
                        dtT[:, mt, c * 512:(c + 1) * 512], spe[:], AF.Ln, bias=1.0)

            for pt in range(NPT):
                nc.vector.tensor_mul(uT[:, pt, :], dtT[:, pt, :],
                                     xc_own[:, pt, :].bitcast(F32))
                nc.vector.tensor_scalar_mul(y_acc[:, pt, :],
                                            xc_own[:, pt, :].bitcast(F32),
                                            sb_D[:, pt, :])

        # ---------- Phase C: selective scan ----------
        with tc.tile_pool(name="pC", bufs=2) as pC, \
             tc.tile_pool(name="pC1", bufs=1) as pC1, \
             tc.tile_pool(name="psC", bufs=1, space="PSUM") as psC:
            hs = [None] * NPT
            for s in range(D_STATE):
                sel_s = pC.tile([32, 2, 128], F32R, tag="sel")
                nc.sync.dma_start(sel_s[:], sel[:, :, s, :])
                for th in range(2):  # token halves
                    off = th * HSEQ
                    B_bc = psC.tile([128, HSEQ], F32, tag="bbc")
                    C_bc = psC.tile([128, HSEQ], F32, tag="cbc")
                    for j in range(2):
                        nc.tensor.matmul(
                            B_bc[:, j * 512:(j + 1) * 512], sel_s[:, 0, :],
                            dbcBC[:, off + j * 512: off + (j + 1) * 512],
                            start=True, stop=True)
                        nc.tensor.matmul(
                            C_bc[:, j * 512:(j + 1) * 512], sel_s[:, 1, :],
                            dbcBC[:, off + j * 512: off + (j + 1) * 512],
                            start=True, stop=True)
                    for pt in range(NPT):
                        dA = pC.tile([128, HSEQ], F32, tag="dA")
                        nc.scalar.activation(dA[:], dtT[:, pt, off:off + HSEQ], AF.Exp,
                                             scale=sb_A[:, pt, s:s + 1])
                        dBu = pC.tile([128, HSEQ], F32, tag="dBu")
                        nc.vector.tensor_mul(dBu[:], uT[:, pt, off:off + HSEQ], B_bc[:])
                        if th == 0:
                            hst = pC1.tile([128, SEQ], F32, tag=f"h{pt}")
                            hs[pt] = hst
                            init = 0.0
                        else:
                            hst = hs[pt]
                            init = hst[:, HSEQ - 1:HSEQ]
                        nc.vector.tensor_tensor_scan(hst[:, off:off + HSEQ],
                                                     dA[:], dBu[:], init,
                                                     OP.mult, OP.add)
                        hc = pC1.tile([128, HSEQ], F32, tag="hc")
                        nc.vector.tensor_mul(hc[:], hst[:, off:off + HSEQ], C_bc[:])
                        nc.vector.tensor_add(y_acc[:, pt, off:off + HSEQ],
                                             y_acc[:, pt, off:off + HSEQ], hc[:])

        # ---------- Phase D: gate + out_proj ----------
        with tc.tile_pool(name="pD", bufs=1) as pD, \
             tc.tile_pool(name="pDo", bufs=2) as pDo:
            sb_wout = pD.tile([128, NPT, DIM], F32R)
            nc.sync.dma_start(sb_wout[:], w_out[:])
            y_g = pD.tile([128, NPT, SEQ], F32R)
            for pt in range(NPT):
                nc.vector.tensor_mul(y_g[:, pt, :], y_acc[:, pt, :], gT[:, pt, :])
            for mt in range(NPT):
                for c in range(NC_):
                    ps4 = psA.tile([128, 512], F32, tag="mm")
                    for k in range(NPT):
                        nc.tensor.matmul(
                            ps4[:], sb_wout[:, k, mt * 128:(mt + 1) * 128],
                            y_g[:, k, c * 512:(c + 1) * 512],
                            start=(k == 0), stop=(k == NPT - 1))
                    ot = pDo.tile([128, 512], F32, tag="ot")
                    nc.scalar.activation(ot[:], ps4[:], AF.Copy)
                    nc.sync.dma_start(oT[:, mt, c * 512:(c + 1) * 512], ot[:])

    nc.compile()
    _PROG_CACHE["nc"] = nc
    return nc


def _prep_core_inputs(x, params, direction, batch, half):
    in_w, conv_w, conv_b, xproj_w, dt_w, dt_b, A_log, D, out_w = params
    xb = x[batch]
    if direction == 1:
        xb = xb[::-1]
    xT = np.ascontiguousarray(xb.T)

    own = np.arange(half * HALF, (half + 1) * HALF)
    other = np.arange((1 - half) * HALF, (2 - half) * HALF)
    perm = np.concatenate([own, other])

    w_in = np.concatenate([in_w[perm], in_w[D_INNER + own]], axis=0).T  # [512, 1536]
    cw = conv_w[perm, 0, :]
    convdiag = np.zeros((128, D_CONV, NFT, 128), np.float32)
    for k in range(D_CONV):
        for m in range(NFT):
            convdiag[:, k, m, :] = np.diag(cw[m * 128:(m + 1) * 128, k])
    convb_ = conv_b[perm].reshape(NFT, 128, 1).transpose(1, 0, 2)
    w_xp = xproj_w[:, perm].T.reshape(NFT, 128, -1).transpose(1, 0, 2)
    # selector matrices: B_s = row s, C_s = row 16+s of the BC block
    sel_ = np.zeros((32, 2, D_STATE, 128), np.float32)
    for s in range(D_STATE):
        sel_[s, 0, s, :] = 1.0
        sel_[D_STATE + s, 1, s, :] = 1.0
    w_dt_ = np.ascontiguousarray(dt_w[own].T)
    dtb_ = dt_b[own].reshape(NPT, 128, 1).transpose(1, 0, 2)
    Acol_ = (-np.exp(A_log[own])).reshape(NPT, 128, D_STATE).transpose(1, 0, 2)
    Dv_ = D[own].reshape(NPT, 128, 1).transpose(1, 0, 2)
    w_out_ = out_w[:, own].T.reshape(NPT, 128, DIM).transpose(1, 0, 2)

    def c32(a):
        return np.ascontiguousarray(a, dtype=np.float32)

    return {
        "xT": c32(xT.reshape(4, 128, SEQ).transpose(1, 0, 2)),
        "w_in": c32(w_in.reshape(4, 128, -1).transpose(1, 0, 2)),
        "convdiag": c32(convdiag),
        "convb": c32(convb_),
        "w_xp": c32(w_xp),
        "sel": c32(sel_),
        "w_dt": c32(w_dt_),
        "dtb": c32(dtb_),
        "Acol": c32(Acol_),
        "Dv": c32(Dv_),
        "w_out": c32(w_out_),
        "zero3": np.zeros((128, 3), np.float32),
    }


def _run(nc, in_maps):
    if os.environ.get("BIMAMBA_SIM"):
        from concourse.bass_interp import CoreSim
        results = []
        n = int(os.environ.get("BIMAMBA_SIM_CORES", "8"))
        for m in in_maps[:n]:
            sim = CoreSim(nc)
            for k, v in m.items():
                sim.tensor(k)[:] = v
            sim.simulate()
            results.append({"oT": np.array(sim.tensor("oT"))})
        return results
    return run_bass_kernel_spmd(nc, in_maps, core_ids=list(range(8))).results


def kernel(**inputs):
    x = np.asarray(inputs["x"], np.float32)
    names = ["in_w", "conv_w", "conv_b", "xproj_w", "dt_w", "dt_b", "A_log", "D", "out_w"]
    fp = tuple(np.asarray(inputs["f_" + n], np.float32) for n in names)
    rp = tuple(np.asarray(inputs["r_" + n], np.float32) for n in names)

    nc = _build_program()
    in_maps = []
    meta = []
    for d in (0, 1):
        for b in range(B_SZ):
            for h in (0, 1):
                in_maps.append(_prep_core_inputs(x, fp if d == 0 else rp, d, b, h))
                meta.append((d, b, h))
    results = _run(nc, in_maps)

    acc = np.zeros((2, B_SZ, SEQ, DIM), np.float32)
    for (d, b, h), r in zip(meta, results):
        oTv = r["oT"]
        o = oTv.transpose(1, 0, 2).reshape(DIM, SEQ).T
        if d == 1:
            o = o[::-1]
        acc[d, b] += o
    out = 0.5 * (acc[0] + acc[1])
    return out.astype(np.float32)


# revision 31
# speedup vs baseline: 1.0001x; 1.0001x over previous
"""BiMamba Trainium2 kernel.

Sharding: 8 cores = (direction f/r) x (batch 2) x (d_inner half 2), SPMD
(one program, per-core data).  The host permutes channel order so each
core's own 512 scan channels occupy positions 0..511; xi/conv are computed
for all 1024 channels on every core (x_proj needs the full d_inner
contraction) with the other half's x_proj contribution accumulated into
PSUM on the fly; z/dt/scan/out_proj cover only the own half.  Partial
out_proj results are summed on the host; the reverse direction is flipped
on the host.

Device pipeline (feature-major [feature, token] layouts, f32r matmuls):
  A) in_proj -> xi; depthwise conv as 4 accumulated diag matmuls;
     silu via sigmoid*x on ScalarE+VectorE; x_proj accumulated over all 8
     channel tiles; z -> silu -> gT
  B) x_proj psum -> dt_raw (f32r) and B/C rows (bf16); dt_proj ->
     softplus(ln(1+exp)) -> dtT (bf16); u = dt*xc (bf16)
  C) selective scan, per (pt pair, state s): broadcast B_s/C_s rows to 128
     partitions via partition-step-0 DMA (bf16); per channel tile:
     dA = exp(A_s*dt) on ScalarE (f32r), dBu = u*B_bc (bf16 2x),
     full-length tensor_tensor_scan on VectorE (fp32 state), hc = h*C_bc
     (bf16 2x), and accumulate y = D*xc + sum_s hc via identity/diag
     matmuls into PSUM (TensorE does the adds)
  D) y_gated = y_psum * silu(z) -> f32r; out_proj partial -> DRAM from PSUM
"""
import os
from contextlib import ExitStack

import numpy as np

import concourse.bacc as bacc
import concourse.bass as bass
import concourse.tile as tile
from concourse import mybir
from concourse.bass_utils import run_bass_kernel_spmd

F32 = mybir.dt.float32
BF16 = mybir.dt.bfloat16
F32R = mybir.dt.float32r
AF = mybir.ActivationFunctionType
OP = mybir.AluOpType
NPBF16 = mybir.dt.np(mybir.dt.bfloat16)

DIM = 512
D_STATE = 16
D_CONV = 4
D_INNER = 1024
DT_RANK = 32
B_SZ = 2
SEQ = 2048
HALF = 512
NPT = HALF // 128     # 4 own-channel partition tiles
NFT = D_INNER // 128  # 8 full-channel partition tiles
NC_ = SEQ // 512      # 4 token chunks
NXD = DT_RANK + 2 * D_STATE  # 64

_PROG_CACHE = {}


def _build_program():
    if "nc" in _PROG_CACHE:
        return _PROG_CACHE["nc"]
    nc = bacc.Bacc("TRN2", target_bir_lowering=False, debug=False)

    xT = nc.dram_tensor("xT", [128, 4, SEQ], F32R, kind="ExternalInput")
    w_in = nc.dram_tensor("w_in", [128, 4, D_INNER + HALF], F32R, kind="ExternalInput")
    convdiag = nc.dram_tensor("convdiag", [128, D_CONV, NFT, 128], F32R, kind="ExternalInput")
    convb = nc.dram_tensor("convb", [128, NFT, 1], F32, kind="ExternalInput")
    w_xp = nc.dram_tensor("w_xp", [128, NFT, NXD], F32R, kind="ExternalInput")
    w_dt = nc.dram_tensor("w_dt", [DT_RANK, HALF], F32R, kind="ExternalInput")
    dtb = nc.dram_tensor("dtb", [128, NPT, 1], F32, kind="ExternalInput")
    Acol = nc.dram_tensor("Acol", [128, NPT, D_STATE], F32, kind="ExternalInput")
    diagD = nc.dram_tensor("diagD", [128, NPT, 128], F32R, kind="ExternalInput")
    ident = nc.dram_tensor("ident", [128, 128], BF16, kind="ExternalInput")
    w_out = nc.dram_tensor("w_out", [128, NPT, DIM], F32R, kind="ExternalInput")
    zero3 = nc.dram_tensor("zero3", [128, 3], F32R, kind="ExternalInput")
    oT = nc.dram_tensor("oT", [128, 4, SEQ], F32, kind="ExternalOutput")

    with tile.TileContext(nc) as tc, ExitStack() as est:
        pP = est.enter_context(tc.tile_pool(name="pP", bufs=1))
        pDram = est.enter_context(tc.tile_pool(name="pDram", bufs=1, space="DRAM"))
        bcd = pDram.tile([2 * D_STATE, SEQ], BF16)

        gT = pP.tile([128, NPT, SEQ], F32)        # silu(z), own half
        xc_own = pP.tile([128, NPT, SEQ], F32R)   # silu(conv(xi)), own half
        dbc_raw = pP.tile([DT_RANK, SEQ], F32R)   # dt_raw rows
        bcb = pP.tile([2 * D_STATE, SEQ], BF16)   # rows 0..15 = B, 16..31 = C

        with tc.tile_pool(name="psX", bufs=4, space="PSUM") as psX:
            psx = []
            for _c in range(NC_):
                psx_t = psX.tile([NXD, 512], F32, tag="xp")
                psx.append(psx_t)

            # ---------- Phase A ----------
            with tc.tile_pool(name="pA", bufs=1) as pA, \
                 tc.tile_pool(name="pAw", bufs=2) as pAw, \
                 tc.tile_pool(name="pXi", bufs=2) as pXi, \
                 tc.tile_pool(name="psA", bufs=3, space="PSUM") as psA:
                sb_xT = pA.tile([128, 4, SEQ], F32R)
                sb_cb = pA.tile([128, NFT, 1], F32)
                sb_wxp = pA.tile([128, NFT, NXD], F32R)
                nc.sync.dma_start(sb_xT[:], xT[:])
                nc.sync.dma_start(sb_cb[:], convb[:])
                nc.sync.dma_start(sb_wxp[:], w_xp[:])

                for m in range(12):
                    win_m = pAw.tile([128, 4, 128], F32R, tag="win")
                    nc.sync.dma_start(win_m[:], w_in[:, :, m * 128:(m + 1) * 128])
                    xi_pad = None
                    if m < 8:
                        xi_pad = pXi.tile([128, 3 + SEQ], F32R, tag="xi_pad")
                        nc.sync.dma_start(xi_pad[:, 0:3], zero3[:])
                    for c in range(NC_):
                        ps = psA.tile([128, 512], F32, tag="mm")
                        for k in range(4):
                            nc.tensor.matmul(
                                ps[:], win_m[:, k, :],
                                sb_xT[:, k, c * 512:(c + 1) * 512],
                                start=(k == 0), stop=(k == 3))
                        if m < 8:
                            nc.scalar.activation(
                                xi_pad[:, 3 + c * 512: 3 + (c + 1) * 512], ps[:], AF.Copy)
                        else:
                            # silu(z) = z * sigmoid(z)
                            sgz = pXi.tile([128, 512], F32, tag="sgz")
                            nc.scalar.activation(sgz[:], ps[:], AF.Sigmoid)
                            nc.vector.tensor_mul(
                                gT[:, m - 8, c * 512:(c + 1) * 512], ps[:], sgz[:])
                    if m < 8:
                        cd_m = pAw.tile([128, D_CONV, 128], F32R, tag="cd")
                        nc.sync.dma_start(cd_m[:], convdiag[:, :, m, :])
                        for c in range(NC_):
                            ps2 = psA.tile([128, 512], F32, tag="mm")
                            for k in range(D_CONV):
                                nc.tensor.matmul(
                                    ps2[:], cd_m[:, k, :],
                                    xi_pad[:, c * 512 + k: c * 512 + k + 512],
                                    start=(k == 0), stop=(k == D_CONV - 1))
                            if m < NPT:
                                xco = xc_own[:, m, c * 512:(c + 1) * 512]
                            else:
                                xco_t = pXi.tile([128, 512], F32R, tag="xco")
                                xco = xco_t[:]
                            # silu(v) = sigmoid(v) * v, v = conv psum + bias
                            sgc = pXi.tile([128, 512], F32, tag="sgc")
                            nc.scalar.activation(sgc[:], ps2[:], AF.Sigmoid,
                                                 bias=sb_cb[:, m, :])
                            nc.vector.scalar_tensor_tensor(
                                xco, ps2[:], sb_cb[:, m, :], sgc[:],
                                OP.add, OP.mult)
                            # accumulate x_proj contribution of this tile
                            nc.tensor.matmul(
                                psx[c][:], sb_wxp[:, m, :], xco,
                                start=(m == 0), stop=(m == 7))

            # unload x_proj accumulators (still inside psX scope)
            for c in range(NC_):
                nc.vector.tensor_copy(dbc_raw[:, c * 512:(c + 1) * 512],
                                      psx[c][0:DT_RANK, :])
                nc.vector.tensor_copy(bcb[:, c * 512:(c + 1) * 512],
                                      psx[c][DT_RANK:NXD, :])
        # stage B/C rows in DRAM so the per-s broadcast DMA can use a
        # partition-step-0 source (SBUF sources reject it)
        nc.sync.dma_start(bcd[:], bcb[:])

        # ---------- Phase B ----------
        pBCD = est.enter_context(tc.tile_pool(name="pBCD", bufs=1))
        dtT = pBCD.tile([128, NPT, SEQ], BF16)
        uT = pBCD.tile([128, NPT, SEQ], BF16)
        sb_A = pBCD.tile([128, NPT, D_STATE], F32)
        sb_dD = pBCD.tile([128, NPT, 128], F32R)
        sb_id = pBCD.tile([128, 128], BF16)
        y_g = pBCD.tile([128, NPT, SEQ], F32R)
        nc.sync.dma_start(sb_A[:], Acol[:])
        nc.sync.dma_start(sb_dD[:], diagD[:])
        nc.sync.dma_start(sb_id[:], ident[:])

        with tc.tile_pool(name="pB", bufs=1) as pB, \
             tc.tile_pool(name="pBt", bufs=2) as pBt, \
             tc.tile_pool(name="psB", bufs=2, space="PSUM") as psB:
            sb_wdt = pB.tile([DT_RANK, HALF], F32R)
            sb_dtb = pB.tile([128, NPT, 1], F32)
            nc.sync.dma_start(sb_wdt[:], w_dt[:])
            nc.sync.dma_start(sb_dtb[:], dtb[:])
            for mt in range(NPT):
                for c in range(NC_):
                    ps3 = psB.tile([128, 512], F32, tag="mm")
                    nc.tensor.matmul(
                        ps3[:], sb_wdt[:, mt * 128:(mt + 1) * 128],
                        dbc_raw[:, c * 512:(c + 1) * 512], start=True, stop=True)
                    # softplus(w) = ln(1 + exp(w)); w = psum + dt_bias
                    spe = pBt.tile([128, 512], F32, tag="spe")
                    nc.scalar.activation(spe[:], ps3[:], AF.Exp, bias=sb_dtb[:, mt, :])
                    nc.scalar.activation(
                        dtT[:, mt, c * 512:(c + 1) * 512], spe[:], AF.Ln, bias=1.0)

            for pt in range(NPT):
                nc.vector.tensor_mul(uT[:, pt, :], dtT[:, pt, :],
                                     xc_own[:, pt, :].bitcast(F32))

        # ---------- Phase C: selective scan ----------
        with tc.tile_pool(name="pC", bufs=2) as pC, \
             tc.tile_pool(name="psC", bufs=8, space="PSUM") as psC:
            for pair in range(2):
                pts = (2 * pair, 2 * pair + 1)
                # y accumulators: one PSUM bank per (pt-in-pair, token chunk)
                yps = {}
                for ptl, pt in enumerate(pts):
                    for q in range(NC_):
                        yps_t = psC.tile([128, 512], F32, tag="yps")
                        yps[(ptl, q)] = yps_t
                        # initialize with D * xc via diag matmul
                        nc.tensor.matmul(
                            yps_t[:], sb_dD[:, pt, :],
                            xc_own[:, pt, q * 512:(q + 1) * 512],
                            start=True, stop=False, skip_group_check=True)
                for s in range(D_STATE):
                    B_bc = pC.tile([128, SEQ], BF16, tag="bbc")
                    C_bc = pC.tile([128, SEQ], BF16, tag="cbc")
                    brow = bcd[s:s + 1, :]
                    crow = bcd[D_STATE + s:D_STATE + s + 1, :]
                    nc.gpsimd.dma_start(B_bc[:], bass.AP(
                        tensor=brow.tensor, offset=brow.offset,
                        ap=[[0, 128]] + list(brow.ap[1:])))
                    nc.gpsimd.dma_start(C_bc[:], bass.AP(
                        tensor=crow.tensor, offset=crow.offset,
                        ap=[[0, 128]] + list(crow.ap[1:])))
                    # high-s iterations run fully on GpSimd to offload VectorE
                    eng = nc.gpsimd if s >= 11 else nc.vector
                    for ptl, pt in enumerate(pts):
                        dA = pC.tile([128, SEQ], F32R, tag="dA")
                        nc.scalar.activation(dA[:], dtT[:, pt, :], AF.Exp,
                                             scale=sb_A[:, pt, s:s + 1])
                        dBu = pC.tile([128, SEQ], BF16, tag="dBu")
                        eng.tensor_mul(dBu[:], uT[:, pt, :], B_bc[:])
                        h = pC.tile([128, SEQ], BF16, tag="h")
                        eng.tensor_tensor_scan(h[:], dA[:], dBu[:], 0.0,
                                               OP.mult, OP.add)
                        hc = pC.tile([128, SEQ], BF16, tag="hc")
                        eng.tensor_mul(hc[:], h[:], C_bc[:])
                        for q in range(NC_):
                            nc.tensor.matmul(
                                yps[(ptl, q)][:], sb_id[:],
                                hc[:, q * 512:(q + 1) * 512],
                                start=False, stop=(s == D_STATE - 1),
                                skip_group_check=True)
                # gate: y_g = y * silu(z)
                for ptl, pt in enumerate(pts):
                    for q in range(NC_):
                        nc.vector.tensor_mul(
                            y_g[:, pt, q * 512:(q + 1) * 512],
                            yps[(ptl, q)][:],
                            gT[:, pt, q * 512:(q + 1) * 512])

        # ---------- Phase D: out_proj ----------
        with tc.tile_pool(name="pD", bufs=1) as pD, \
             tc.tile_pool(name="pDo", bufs=3) as pDo, \
             tc.tile_pool(name="psD", bufs=3, space="PSUM") as psD:
            sb_wout = pD.tile([128, NPT, DIM], F32R)
            nc.sync.dma_start(sb_wout[:], w_out[:])
            for mt in range(NPT):
                for c in range(NC_):
                    ps4 = psD.tile([128, 512], F32, tag="mm")
                    for k in range(NPT):
                        nc.tensor.matmul(
                            ps4[:], sb_wout[:, k, mt * 128:(mt + 1) * 128],
                            y_g[:, k, c * 512:(c + 1) * 512],
                            start=(k == 0), stop=(k == NPT - 1))
                    ot = pDo.tile([128, 512], F32, tag="ot")
                    nc.scalar.activation(ot[:], ps4[:], AF.Copy)
                    nc.sync.dma_start(oT[:, mt, c * 512:(c + 1) * 512], ot[:])

    nc.compile()
    _PROG_CACHE["nc"] = nc
    return nc


def _prep_core_inputs(x, params, direction, batch, half):
    in_w, conv_w, conv_b, xproj_w, dt_w, dt_b, A_log, D, out_w = params
    xb = x[batch]
    if direction == 1:
        xb = xb[::-1]
    xT = np.ascontiguousarray(xb.T)

    own = np.arange(half * HALF, (half + 1) * HALF)
    other = np.arange((1 - half) * HALF, (2 - half) * HALF)
    perm = np.concatenate([own, other])

    w_in = np.concatenate([in_w[perm], in_w[D_INNER + own]], axis=0).T  # [512, 1536]
    cw = conv_w[perm, 0, :]                                            # [1024, 4]
    convdiag = np.zeros((128, D_CONV, NFT, 128), np.float32)
    ii = np.arange(128)
    for k in range(D_CONV):
        for m in range(NFT):
            convdiag[ii, k, m, ii] = cw[m * 128:(m + 1) * 128, k]
    convb_ = conv_b[perm].reshape(NFT, 128, 1).transpose(1, 0, 2)
    w_xp = xproj_w[:, perm].T.reshape(NFT, 128, -1).transpose(1, 0, 2)
    w_dt_ = np.ascontiguousarray(dt_w[own].T)
    dtb_ = dt_b[own].reshape(NPT, 128, 1).transpose(1, 0, 2)
    Acol_ = (-np.exp(A_log[own])).reshape(NPT, 128, D_STATE).transpose(1, 0, 2)
    dD = np.zeros((128, NPT, 128), np.float32)
    Dr = D[own].reshape(NPT, 128)
    for ptn in range(NPT):
        dD[ii, ptn, ii] = Dr[ptn]
    w_out_ = out_w[:, own].T.reshape(NPT, 128, DIM).transpose(1, 0, 2)

    def c32(a):
        return np.ascontiguousarray(a, dtype=np.float32)

    return {
        "xT": c32(xT.reshape(4, 128, SEQ).transpose(1, 0, 2)),
        "w_in": c32(w_in.reshape(4, 128, -1).transpose(1, 0, 2)),
        "convdiag": c32(convdiag),
        "convb": c32(convb_),
        "w_xp": c32(w_xp),
        "w_dt": c32(w_dt_),
        "dtb": c32(dtb_),
        "Acol": c32(Acol_),
        "diagD": c32(dD),
        "ident": np.eye(128, dtype=NPBF16),
        "w_out": c32(w_out_),
        "zero3": np.zeros((128, 3), np.float32),
    }


def _run(nc, in_maps):
    if os.environ.get("BIMAMBA_SIM"):
        from concourse.bass_interp import CoreSim
        results = []
        n = int(os.environ.get("BIMAMBA_SIM_CORES", "8"))
        for m in in_maps[:n]:
            sim = CoreSim(nc)
            for k, v in m.items():
                sim.tensor(k)[:] = v
            sim.simulate()
            results.append({"oT": np.array(sim.tensor("oT"))})
        return results
    return run_bass_kernel_spmd(nc, in_maps, core_ids=list(range(8))).results


def _prep_all(inputs):
    x = np.asarray(inputs["x"], np.float32)
    names = ["in_w", "conv_w", "conv_b", "xproj_w", "dt_w", "dt_b", "A_log", "D", "out_w"]
    fp = tuple(np.asarray(inputs["f_" + n], np.float32) for n in names)
    rp = tuple(np.asarray(inputs["r_" + n], np.float32) for n in names)
    in_maps = []
    meta = []
    for d in (0, 1):
        for b in range(B_SZ):
            for h in (0, 1):
                in_maps.append(_prep_core_inputs(x, fp if d == 0 else rp, d, b, h))
                meta.append((d, b, h))
    return in_maps, meta


def kernel(**inputs):
    nc = _build_program()
    in_maps, meta = _prep_all(inputs)
    results = _run(nc, in_maps)

    acc = np.zeros((2, B_SZ, SEQ, DIM), np.float32)
    for (d, b, h), r in zip(meta, results):
        oTv = r["oT"]
        o = oTv.transpose(1, 0, 2).reshape(DIM, SEQ).T
        if d == 1:
            o = o[::-1]
        acc[d, b] += o
    out = 0.5 * (acc[0] + acc[1])
    return out.astype(np.float32)


# revision 33
# speedup vs baseline: 6156.8509x; 6156.2643x over previous
"""BiMamba Trainium2 kernel.

Sharding: 8 cores = (direction f/r) x (batch 2) x (d_inner half 2), SPMD
(one program, per-core data).  The host permutes channel order so each
core's own 512 scan channels occupy positions 0..511; xi/conv are computed
for all 1024 channels on every core (x_proj needs the full d_inner
contraction) with the other half's x_proj contribution accumulated into
PSUM on the fly; z/dt/scan/out_proj cover only the own half.  Partial
out_proj results are summed on the host; the reverse direction is flipped
on the host.

Device pipeline (feature-major [feature, token] layouts, f32r matmuls):
  A) in_proj -> xi; depthwise conv as 4 accumulated diag matmuls;
     silu via sigmoid*x on ScalarE+VectorE; x_proj accumulated over all 8
     channel tiles; z -> silu -> gT
  B) x_proj psum -> dt_raw (f32r) and B/C rows (bf16); dt_proj ->
     softplus(ln(1+exp)) -> dtT (bf16); u = dt*xc (bf16)
  C) selective scan, per (pt pair, state s): broadcast B_s/C_s rows to 128
     partitions via partition-step-0 DMA (bf16); per channel tile:
     dA = exp(A_s*dt) on ScalarE (f32r), dBu = u*B_bc (bf16 2x),
     full-length tensor_tensor_scan on VectorE (fp32 state), hc = h*C_bc
     (bf16 2x), and accumulate y = D*xc + sum_s hc via identity/diag
     matmuls into PSUM (TensorE does the adds)
  D) y_gated = y_psum * silu(z) -> f32r; out_proj partial -> DRAM from PSUM
"""
import os
from contextlib import ExitStack

import numpy as np

import concourse.bacc as bacc
import concourse.bass as bass
import concourse.tile as tile
from concourse import mybir
from concourse.bass_utils import run_bass_kernel_spmd

F32 = mybir.dt.float32
BF16 = mybir.dt.bfloat16
F32R = mybir.dt.float32r
AF = mybir.ActivationFunctionType
OP = mybir.AluOpType
NPBF16 = mybir.dt.np(mybir.dt.bfloat16)

DIM = 512
D_STATE = 16
D_CONV = 4
D_INNER = 1024
DT_RANK = 32
B_SZ = 2
SEQ = 2048
HALF = 512
NPT = HALF // 128     # 4 own-channel partition tiles
NFT = D_INNER // 128  # 8 full-channel partition tiles
NC_ = SEQ // 512      # 4 token chunks
NXD = DT_RANK + 2 * D_STATE  # 64

_PROG_CACHE = {}


def _build_program():
    if "nc" in _PROG_CACHE:
        return _PROG_CACHE["nc"]
    nc = bacc.Bacc("TRN2", target_bir_lowering=False, debug=False)

    xT = nc.dram_tensor("xT", [128, 4, SEQ], F32R, kind="ExternalInput")
    w_in = nc.dram_tensor("w_in", [128, 4, D_INNER + HALF], F32R, kind="ExternalInput")
    convdiag = nc.dram_tensor("convdiag", [128, D_CONV, NFT, 128], F32R, kind="ExternalInput")
    convb = nc.dram_tensor("convb", [128, NFT, 1], F32, kind="ExternalInput")
    w_xp = nc.dram_tensor("w_xp", [128, NFT, NXD], F32R, kind="ExternalInput")
    w_dt = nc.dram_tensor("w_dt", [DT_RANK, HALF], F32R, kind="ExternalInput")
    dtb = nc.dram_tensor("dtb", [128, NPT, 1], F32, kind="ExternalInput")
    Acol = nc.dram_tensor("Acol", [128, NPT, D_STATE], F32, kind="ExternalInput")
    diagD = nc.dram_tensor("diagD", [128, NPT, 128], F32R, kind="ExternalInput")
    ident = nc.dram_tensor("ident", [128, 128], BF16, kind="ExternalInput")
    w_out = nc.dram_tensor("w_out", [128, NPT, DIM], F32R, kind="ExternalInput")
    zero3 = nc.dram_tensor("zero3", [128, 3], F32R, kind="ExternalInput")
    oT = nc.dram_tensor("oT", [128, 4, SEQ], F32, kind="ExternalOutput")

    loop_n = int(os.environ.get("BIMAMBA_LOOP", "0"))
    with tile.TileContext(nc) as tc, ExitStack() as est:
        if loop_n > 1:
            est.enter_context(tc.For_i(0, loop_n, 1))
        pP = est.enter_context(tc.tile_pool(name="pP", bufs=1))
        pDram = est.enter_context(tc.tile_pool(name="pDram", bufs=1, space="DRAM"))
        bcd = pDram.tile([2 * D_STATE, SEQ], BF16)

        gT = pP.tile([128, NPT, SEQ], F32)        # silu(z), own half
        xc_own = pP.tile([128, NPT, SEQ], F32R)   # silu(conv(xi)), own half
        dbc_raw = pP.tile([DT_RANK, SEQ], F32R)   # dt_raw rows
        bcb = pP.tile([2 * D_STATE, SEQ], BF16)   # rows 0..15 = B, 16..31 = C

        with tc.tile_pool(name="psX", bufs=4, space="PSUM") as psX:
            psx = []
            for _c in range(NC_):
                psx_t = psX.tile([NXD, 512], F32, tag="xp")
                psx.append(psx_t)

            # ---------- Phase A ----------
            with tc.tile_pool(name="pA", bufs=1) as pA, \
                 tc.tile_pool(name="pAw", bufs=2) as pAw, \
                 tc.tile_pool(name="pXi", bufs=2) as pXi, \
                 tc.tile_pool(name="psA", bufs=3, space="PSUM") as psA:
                sb_xT = pA.tile([128, 4, SEQ], F32R)
                sb_cb = pA.tile([128, NFT, 1], F32)
                sb_wxp = pA.tile([128, NFT, NXD], F32R)
                nc.sync.dma_start(sb_xT[:], xT[:])
                nc.sync.dma_start(sb_cb[:], convb[:])
                nc.sync.dma_start(sb_wxp[:], w_xp[:])

                for m in range(12):
                    win_m = pAw.tile([128, 4, 128], F32R, tag="win")
                    nc.sync.dma_start(win_m[:], w_in[:, :, m * 128:(m + 1) * 128])
                    xi_pad = None
                    if m < 8:
                        xi_pad = pXi.tile([128, 3 + SEQ], F32R, tag="xi_pad")
                        nc.sync.dma_start(xi_pad[:, 0:3], zero3[:])
                    for c in range(NC_):
                        ps = psA.tile([128, 512], F32, tag="mm")
                        for k in range(4):
                            nc.tensor.matmul(
                                ps[:], win_m[:, k, :],
                                sb_xT[:, k, c * 512:(c + 1) * 512],
                                start=(k == 0), stop=(k == 3))
                        if m < 8:
                            nc.scalar.activation(
                                xi_pad[:, 3 + c * 512: 3 + (c + 1) * 512], ps[:], AF.Copy)
                        else:
                            # silu(z) = z * sigmoid(z)
                            sgz = pXi.tile([128, 512], F32, tag="sgz")
                            nc.scalar.activation(sgz[:], ps[:], AF.Sigmoid)
                            nc.vector.tensor_mul(
                                gT[:, m - 8, c * 512:(c + 1) * 512], ps[:], sgz[:])
                    if m < 8:
                        cd_m = pAw.tile([128, D_CONV, 128], F32R, tag="cd")
                        nc.sync.dma_start(cd_m[:], convdiag[:, :, m, :])
                        for c in range(NC_):
                            ps2 = psA.tile([128, 512], F32, tag="mm")
                            for k in range(D_CONV):
                                nc.tensor.matmul(
                                    ps2[:], cd_m[:, k, :],
                                    xi_pad[:, c * 512 + k: c * 512 + k + 512],
                                    start=(k == 0), stop=(k == D_CONV - 1))
                            if m < NPT:
                                xco = xc_own[:, m, c * 512:(c + 1) * 512]
                            else:
                                xco_t = pXi.tile([128, 512], F32R, tag="xco")
                                xco = xco_t[:]
                            # silu(v) = sigmoid(v) * v, v = conv psum + bias
                            sgc = pXi.tile([128, 512], F32, tag="sgc")
                            nc.scalar.activation(sgc[:], ps2[:], AF.Sigmoid,
                                                 bias=sb_cb[:, m, :])
                            nc.vector.scalar_tensor_tensor(
                                xco, ps2[:], sb_cb[:, m, :], sgc[:],
                                OP.add, OP.mult)
                            # accumulate x_proj contribution of this tile
                            nc.tensor.matmul(
                                psx[c][:], sb_wxp[:, m, :], xco,
                                start=(m == 0), stop=(m == 7))

            # unload x_proj accumulators (still inside psX scope)
            for c in range(NC_):
                nc.vector.tensor_copy(dbc_raw[:, c * 512:(c + 1) * 512],
                                      psx[c][0:DT_RANK, :])
                nc.vector.tensor_copy(bcb[:, c * 512:(c + 1) * 512],
                                      psx[c][DT_RANK:NXD, :])
        # stage B/C rows in DRAM so the per-s broadcast DMA can use a
        # partition-step-0 source (SBUF sources reject it)
        nc.sync.dma_start(bcd[:], bcb[:])

        # ---------- Phase B ----------
        pBCD = est.enter_context(tc.tile_pool(name="pBCD", bufs=1))
        dtT = pBCD.tile([128, NPT, SEQ], BF16)
        uT = pBCD.tile([128, NPT, SEQ], BF16)
        sb_A = pBCD.tile([128, NPT, D_STATE], F32)
        sb_dD = pBCD.tile([128, NPT, 128], F32R)
        sb_id = pBCD.tile([128, 128], BF16)
        y_g = pBCD.tile([128, NPT, SEQ], F32R)
        nc.sync.dma_start(sb_A[:], Acol[:])
        nc.sync.dma_start(sb_dD[:], diagD[:])
        nc.sync.dma_start(sb_id[:], ident[:])

        with tc.tile_pool(name="pB", bufs=1) as pB, \
             tc.tile_pool(name="pBt", bufs=2) as pBt, \
             tc.tile_pool(name="psB", bufs=2, space="PSUM") as psB:
            sb_wdt = pB.tile([DT_RANK, HALF], F32R)
            sb_dtb = pB.tile([128, NPT, 1], F32)
            nc.sync.dma_start(sb_wdt[:], w_dt[:])
            nc.sync.dma_start(sb_dtb[:], dtb[:])
            for mt in range(NPT):
                for c in range(NC_):
                    ps3 = psB.tile([128, 512], F32, tag="mm")
                    nc.tensor.matmul(
                        ps3[:], sb_wdt[:, mt * 128:(mt + 1) * 128],
                        dbc_raw[:, c * 512:(c + 1) * 512], start=True, stop=True)
                    # softplus(w) = ln(1 + exp(w)); w = psum + dt_bias
                    spe = pBt.tile([128, 512], F32, tag="spe")
                    nc.scalar.activation(spe[:], ps3[:], AF.Exp, bias=sb_dtb[:, mt, :])
                    nc.scalar.activation(
                        dtT[:, mt, c * 512:(c + 1) * 512], spe[:], AF.Ln, bias=1.0)

            for pt in range(NPT):
                nc.vector.tensor_mul(uT[:, pt, :], dtT[:, pt, :],
                                     xc_own[:, pt, :].bitcast(F32))

        # ---------- Phase C: selective scan ----------
        with tc.tile_pool(name="pC", bufs=2) as pC, \
             tc.tile_pool(name="psC", bufs=8, space="PSUM") as psC:
            for pair in range(2):
                pts = (2 * pair, 2 * pair + 1)
                # y accumulators: one PSUM bank per (pt-in-pair, token chunk)
                yps = {}
                for ptl, pt in enumerate(pts):
                    for q in range(NC_):
                        yps_t = psC.tile([128, 512], F32, tag="yps")
                        yps[(ptl, q)] = yps_t
                        # initialize with D * xc via diag matmul
                        nc.tensor.matmul(
                            yps_t[:], sb_dD[:, pt, :],
                            xc_own[:, pt, q * 512:(q + 1) * 512],
                            start=True, stop=False, skip_group_check=True)
                for s in range(D_STATE):
                    B_bc = pC.tile([128, SEQ], BF16, tag="bbc")
                    C_bc = pC.tile([128, SEQ], BF16, tag="cbc")
                    brow = bcd[s:s + 1, :]
                    crow = bcd[D_STATE + s:D_STATE + s + 1, :]
                    nc.gpsimd.dma_start(B_bc[:], bass.AP(
                        tensor=brow.tensor, offset=brow.offset,
                        ap=[[0, 128]] + list(brow.ap[1:])))
                    nc.gpsimd.dma_start(C_bc[:], bass.AP(
                        tensor=crow.tensor, offset=crow.offset,
                        ap=[[0, 128]] + list(crow.ap[1:])))
                    # high-s multiplies run on GpSimd to offload VectorE
                    # (the scan opcode itself is VectorE-only)
                    eng = nc.gpsimd if s >= 10 else nc.vector
                    for ptl, pt in enumerate(pts):
                        dA = pC.tile([128, SEQ], F32R, tag="dA")
                        nc.scalar.activation(dA[:], dtT[:, pt, :], AF.Exp,
                                             scale=sb_A[:, pt, s:s + 1])
                        dBu = pC.tile([128, SEQ], BF16, tag="dBu")
                        eng.tensor_mul(dBu[:], uT[:, pt, :], B_bc[:])
                        h = pC.tile([128, SEQ], BF16, tag="h")
                        nc.vector.tensor_tensor_scan(h[:], dA[:], dBu[:], 0.0,
                                                     OP.mult, OP.add)
                        hc = pC.tile([128, SEQ], BF16, tag="hc")
                        eng.tensor_mul(hc[:], h[:], C_bc[:])
                        for q in range(NC_):
                            nc.tensor.matmul(
                                yps[(ptl, q)][:], sb_id[:],
                                hc[:, q * 512:(q + 1) * 512],
                                start=False, stop=(s == D_STATE - 1),
                                skip_group_check=True)
                # gate: y_g = y * silu(z)
                for ptl, pt in enumerate(pts):
                    for q in range(NC_):
                        nc.vector.tensor_mul(
                            y_g[:, pt, q * 512:(q + 1) * 512],
                            yps[(ptl, q)][:],
                            gT[:, pt, q * 512:(q + 1) * 512])

        # ---------- Phase D: out_proj ----------
        with tc.tile_pool(name="pD", bufs=1) as pD, \
             tc.tile_pool(name="pDo", bufs=3) as pDo, \
             tc.tile_pool(name="psD", bufs=3, space="PSUM") as psD:
            sb_wout = pD.tile([128, NPT, DIM], F32R)
            nc.sync.dma_start(sb_wout[:], w_out[:])
            for mt in range(NPT):
                for c in range(NC_):
                    ps4 = psD.tile([128, 512], F32, tag="mm")
                    for k in range(NPT):
                        nc.tensor.matmul(
                            ps4[:], sb_wout[:, k, mt * 128:(mt + 1) * 128],
                            y_g[:, k, c * 512:(c + 1) * 512],
                            start=(k == 0), stop=(k == NPT - 1))
                    ot = pDo.tile([128, 512], F32, tag="ot")
                    nc.scalar.activation(ot[:], ps4[:], AF.Copy)
                    nc.sync.dma_start(oT[:, mt, c * 512:(c + 1) * 512], ot[:])

    nc.compile()
    _PROG_CACHE["nc"] = nc
    return nc


def _prep_core_inputs(x, params, direction, batch, half):
    in_w, conv_w, conv_b, xproj_w, dt_w, dt_b, A_log, D, out_w = params
    xb = x[batch]
    if direction == 1:
        xb = xb[::-1]
    xT = np.ascontiguousarray(xb.T)

    own = np.arange(half * HALF, (half + 1) * HALF)
    other = np.arange((1 - half) * HALF, (2 - half) * HALF)
    perm = np.concatenate([own, other])

    w_in = np.concatenate([in_w[perm], in_w[D_INNER + own]], axis=0).T  # [512, 1536]
    cw = conv_w[perm, 0, :]                                            # [1024, 4]
    convdiag = np.zeros((128, D_CONV, NFT, 128), np.float32)
    ii = np.arange(128)
    for k in range(D_CONV):
        for m in range(NFT):
            convdiag[ii, k, m, ii] = cw[m * 128:(m + 1) * 128, k]
    convb_ = conv_b[perm].reshape(NFT, 128, 1).transpose(1, 0, 2)
    w_xp = xproj_w[:, perm].T.reshape(NFT, 128, -1).transpose(1, 0, 2)
    w_dt_ = np.ascontiguousarray(dt_w[own].T)
    dtb_ = dt_b[own].reshape(NPT, 128, 1).transpose(1, 0, 2)
    Acol_ = (-np.exp(A_log[own])).reshape(NPT, 128, D_STATE).transpose(1, 0, 2)
    dD = np.zeros((128, NPT, 128), np.float32)
    Dr = D[own].reshape(NPT, 128)
    for ptn in range(NPT):
        dD[ii, ptn, ii] = Dr[ptn]
    w_out_ = out_w[:, own].T.reshape(NPT, 128, DIM).transpose(1, 0, 2)

    def c32(a):
        return np.ascontiguousarray(a, dtype=np.float32)

    return {
        "xT": c32(xT.reshape(4, 128, SEQ).transpose(1, 0, 2)),
        "w_in": c32(w_in.reshape(4, 128, -1).transpose(1, 0, 2)),
        "convdiag": c32(convdiag),
        "convb": c32(convb_),
        "w_xp": c32(w_xp),
        "w_dt": c32(w_dt_),
        "dtb": c32(dtb_),
        "Acol": c32(Acol_),
        "diagD": c32(dD),
        "ident": np.eye(128, dtype=NPBF16),
        "w_out": c32(w_out_),
        "zero3": np.zeros((128, 3), np.float32),
    }


def _run(nc, in_maps):
    if os.environ.get("BIMAMBA_SIM"):
        from concourse.bass_interp import CoreSim
        results = []
        n = int(os.environ.get("BIMAMBA_SIM_CORES", "8"))
        for m in in_maps[:n]:
            sim = CoreSim(nc)
            for k, v in m.items():
                sim.tensor(k)[:] = v
            sim.simulate()
            results.append({"oT": np.array(sim.tensor("oT"))})
        return results
    return run_bass_kernel_spmd(nc, in_maps, core_ids=list(range(8))).results


def _prep_all(inputs):
    x = np.asarray(inputs["x"], np.float32)
    names = ["in_w", "conv_w", "conv_b", "xproj_w", "dt_w", "dt_b", "A_log", "D", "out_w"]
    fp = tuple(np.asarray(inputs["f_" + n], np.float32) for n in names)
    rp = tuple(np.asarray(inputs["r_" + n], np.float32) for n in names)
    in_maps = []
    meta = []
    for d in (0, 1):
        for b in range(B_SZ):
            for h in (0, 1):
                in_maps.append(_prep_core_inputs(x, fp if d == 0 else rp, d, b, h))
                meta.append((d, b, h))
    return in_maps, meta


def kernel(**inputs):
    nc = _build_program()
    in_maps, meta = _prep_all(inputs)
    results = _run(nc, in_maps)

    acc = np.zeros((2, B_SZ, SEQ, DIM), np.float32)
    for (d, b, h), r in zip(meta, results):
        oTv = r["oT"]
        o = oTv.transpose(1, 0, 2).reshape(DIM, SEQ).T
        if d == 1:
            o = o[::-1]
        acc[d, b] += o
    out = 0.5 * (acc[0] + acc[1])
    return out.astype(np.float32)


# revision 36
# speedup vs baseline: 6472.6625x; 1.0513x over previous
"""BiMamba Trainium2 kernel.

Sharding: 8 cores = (direction f/r) x (batch 2) x (d_inner half 2), SPMD
(one program, per-core data).  The host permutes channel order so each
core's own 512 scan channels occupy positions 0..511; xi/conv are computed
for all 1024 channels on every core (x_proj needs the full d_inner
contraction) with the other half's x_proj contribution accumulated into
PSUM on the fly; z/dt/scan/out_proj cover only the own half.  Partial
out_proj results are summed on the host; the reverse direction is flipped
on the host.

Device pipeline (feature-major [feature, token] layouts, f32r matmuls):
  A) in_proj -> xi; depthwise conv as 4 accumulated diag matmuls;
     silu via sigmoid*x on ScalarE+VectorE; x_proj accumulated over all 8
     channel tiles; z -> silu -> gT
  B) x_proj psum -> dt_raw (f32r) and B/C rows (bf16); dt_proj ->
     softplus(ln(1+exp)) -> dtT (bf16); u = dt*xc (bf16)
  C) selective scan, per (pt pair, state s): broadcast B_s/C_s rows to 128
     partitions via partition-step-0 DMA (bf16); per channel tile:
     dA = exp(A_s*dt) on ScalarE (f32r), dBu = u*B_bc (bf16 2x),
     full-length tensor_tensor_scan on VectorE (fp32 state), hc = h*C_bc
     (bf16 2x), and accumulate y = D*xc + sum_s hc via identity/diag
     matmuls into PSUM (TensorE does the adds)
  D) y_gated = y_psum * silu(z) -> f32r; out_proj partial -> DRAM from PSUM
"""
import os
from contextlib import ExitStack

import numpy as np

import concourse.bacc as bacc
import concourse.bass as bass
import concourse.tile as tile
from concourse import mybir
from concourse.bass_utils import run_bass_kernel_spmd

F32 = mybir.dt.float32
BF16 = mybir.dt.bfloat16
F32R = mybir.dt.float32r
AF = mybir.ActivationFunctionType
OP = mybir.AluOpType
NPBF16 = mybir.dt.np(mybir.dt.bfloat16)

DIM = 512
D_STATE = 16
D_CONV = 4
D_INNER = 1024
DT_RANK = 32
B_SZ = 2
SEQ = 2048
HALF = 512
NPT = HALF // 128     # 4 own-channel partition tiles
NFT = D_INNER // 128  # 8 full-channel partition tiles
NC_ = SEQ // 512      # 4 token chunks
NXD = DT_RANK + 2 * D_STATE  # 64

_PROG_CACHE = {}


def _build_program():
    if "nc" in _PROG_CACHE:
        return _PROG_CACHE["nc"]
    nc = bacc.Bacc("TRN2", target_bir_lowering=False, debug=False)

    xT = nc.dram_tensor("xT", [128, 4, SEQ], F32R, kind="ExternalInput")
    w_in = nc.dram_tensor("w_in", [128, 4, D_INNER + HALF], F32R, kind="ExternalInput")
    convdiag = nc.dram_tensor("convdiag", [128, D_CONV, NFT, 128], F32R, kind="ExternalInput")
    convb = nc.dram_tensor("convb", [128, NFT, 1], F32, kind="ExternalInput")
    w_xp = nc.dram_tensor("w_xp", [128, NFT, NXD], F32R, kind="ExternalInput")
    w_dt = nc.dram_tensor("w_dt", [DT_RANK, HALF], F32R, kind="ExternalInput")
    dtb = nc.dram_tensor("dtb", [128, NPT, 1], F32, kind="ExternalInput")
    Acol = nc.dram_tensor("Acol", [128, NPT, D_STATE], F32, kind="ExternalInput")
    diagD = nc.dram_tensor("diagD", [128, NPT, 128], F32R, kind="ExternalInput")
    ident = nc.dram_tensor("ident", [128, 128], BF16, kind="ExternalInput")
    w_out = nc.dram_tensor("w_out", [128, NPT, DIM], F32R, kind="ExternalInput")
    zero3 = nc.dram_tensor("zero3", [128, 3], F32R, kind="ExternalInput")
    oT = nc.dram_tensor("oT", [128, 4, SEQ], F32, kind="ExternalOutput")

    loop_n = int(os.environ.get("BIMAMBA_LOOP", "0"))
    with tile.TileContext(nc) as tc, ExitStack() as est:
        if loop_n > 1:
            est.enter_context(tc.For_i(0, loop_n, 1))
        pP = est.enter_context(tc.tile_pool(name="pP", bufs=1))
        pDram = est.enter_context(tc.tile_pool(name="pDram", bufs=1, space="DRAM"))
        bcd = pDram.tile([2 * D_STATE, SEQ], BF16)

        gT = pP.tile([128, NPT, SEQ], F32)        # silu(z), own half
        xc_own = pP.tile([128, NPT, SEQ], F32R)   # silu(conv(xi)), own half
        dbc_raw = pP.tile([DT_RANK, SEQ], F32R)   # dt_raw rows
        bcb = pP.tile([2 * D_STATE, SEQ], BF16)   # rows 0..15 = B, 16..31 = C

        with tc.tile_pool(name="psX", bufs=4, space="PSUM") as psX:
            psx = []
            for _c in range(NC_):
                psx_t = psX.tile([NXD, 512], F32, tag="xp")
                psx.append(psx_t)

            # ---------- Phase A ----------
            with tc.tile_pool(name="pA", bufs=1) as pA, \
                 tc.tile_pool(name="pAw", bufs=2) as pAw, \
                 tc.tile_pool(name="pXi", bufs=2) as pXi, \
                 tc.tile_pool(name="psA", bufs=3, space="PSUM") as psA:
                sb_xT = pA.tile([128, 4, SEQ], F32R)
                sb_cb = pA.tile([128, NFT, 1], F32)
                sb_wxp = pA.tile([128, NFT, NXD], F32R)
                nc.sync.dma_start(sb_xT[:], xT[:])
                nc.sync.dma_start(sb_cb[:], convb[:])
                nc.sync.dma_start(sb_wxp[:], w_xp[:])

                # xi/conv channel tiles first (x_proj finishes earlier so the
                # scan phase can start); z tiles last
                for m in list(range(8)) + list(range(8, 12)):
                    win_m = pAw.tile([128, 4, 128], F32R, tag="win")
                    nc.sync.dma_start(win_m[:], w_in[:, :, m * 128:(m + 1) * 128])
                    xi_pad = None
                    if m < 8:
                        xi_pad = pXi.tile([128, 3 + SEQ], F32R, tag="xi_pad")
                        nc.sync.dma_start(xi_pad[:, 0:3], zero3[:])
                    for c in range(NC_):
                        ps = psA.tile([128, 512], F32, tag="mm")
                        for k in range(4):
                            nc.tensor.matmul(
                                ps[:], win_m[:, k, :],
                                sb_xT[:, k, c * 512:(c + 1) * 512],
                                start=(k == 0), stop=(k == 3))
                        if m < 8:
                            nc.vector.tensor_copy(
                                xi_pad[:, 3 + c * 512: 3 + (c + 1) * 512], ps[:])
                        else:
                            # silu(z) = z * sigmoid(z)
                            sgz = pXi.tile([128, 512], F32, tag="sgz")
                            nc.scalar.activation(sgz[:], ps[:], AF.Sigmoid)
                            nc.vector.tensor_mul(
                                gT[:, m - 8, c * 512:(c + 1) * 512], ps[:], sgz[:])
                    if m < 8:
                        cd_m = pAw.tile([128, D_CONV, 128], F32R, tag="cd")
                        nc.sync.dma_start(cd_m[:], convdiag[:, :, m, :])
                        for c in range(NC_):
                            ps2 = psA.tile([128, 512], F32, tag="mm")
                            for k in range(D_CONV):
                                nc.tensor.matmul(
                                    ps2[:], cd_m[:, k, :],
                                    xi_pad[:, c * 512 + k: c * 512 + k + 512],
                                    start=(k == 0), stop=(k == D_CONV - 1))
                            if m < NPT:
                                xco = xc_own[:, m, c * 512:(c + 1) * 512]
                            else:
                                xco_t = pXi.tile([128, 512], F32R, tag="xco")
                                xco = xco_t[:]
                            # silu(v) = sigmoid(v) * v, v = conv psum + bias
                            sgc = pXi.tile([128, 512], F32, tag="sgc")
                            nc.scalar.activation(sgc[:], ps2[:], AF.Sigmoid,
                                                 bias=sb_cb[:, m, :])
                            nc.vector.scalar_tensor_tensor(
                                xco, ps2[:], sb_cb[:, m, :], sgc[:],
                                OP.add, OP.mult)
                            # accumulate x_proj contribution of this tile
                            nc.tensor.matmul(
                                psx[c][:], sb_wxp[:, m, :], xco,
                                start=(m == 0), stop=(m == 7))

            # unload x_proj accumulators (still inside psX scope)
            for c in range(NC_):
                nc.vector.tensor_copy(dbc_raw[:, c * 512:(c + 1) * 512],
                                      psx[c][0:DT_RANK, :])
                nc.vector.tensor_copy(bcb[:, c * 512:(c + 1) * 512],
                                      psx[c][DT_RANK:NXD, :])
        # stage B/C rows in DRAM so the per-s broadcast DMA can use a
        # partition-step-0 source (SBUF sources reject it)
        nc.sync.dma_start(bcd[:], bcb[:])

        # ---------- Phase B ----------
        pBCD = est.enter_context(tc.tile_pool(name="pBCD", bufs=1))
        dtT = pBCD.tile([128, NPT, SEQ], BF16)
        uT = pBCD.tile([128, NPT, SEQ], BF16)
        sb_A = pBCD.tile([128, NPT, D_STATE], F32)
        sb_dD = pBCD.tile([128, NPT, 128], F32R)
        sb_id = pBCD.tile([128, 128], BF16)
        y_g = pBCD.tile([128, NPT, SEQ], F32R)
        nc.sync.dma_start(sb_A[:], Acol[:])
        nc.sync.dma_start(sb_dD[:], diagD[:])
        nc.sync.dma_start(sb_id[:], ident[:])

        with tc.tile_pool(name="pB", bufs=1) as pB, \
             tc.tile_pool(name="pBt", bufs=2) as pBt, \
             tc.tile_pool(name="psB", bufs=2, space="PSUM") as psB:
            sb_wdt = pB.tile([DT_RANK, HALF], F32R)
            sb_dtb = pB.tile([128, NPT, 1], F32)
            nc.sync.dma_start(sb_wdt[:], w_dt[:])
            nc.sync.dma_start(sb_dtb[:], dtb[:])
            for mt in range(NPT):
                for c in range(NC_):
                    ps3 = psB.tile([128, 512], F32, tag="mm")
                    nc.tensor.matmul(
                        ps3[:], sb_wdt[:, mt * 128:(mt + 1) * 128],
                        dbc_raw[:, c * 512:(c + 1) * 512], start=True, stop=True)
                    # softplus(w) = ln(1 + exp(w)); w = psum + dt_bias
                    spe = pBt.tile([128, 512], F32, tag="spe")
                    nc.scalar.activation(spe[:], ps3[:], AF.Exp, bias=sb_dtb[:, mt, :])
                    nc.scalar.activation(
                        dtT[:, mt, c * 512:(c + 1) * 512], spe[:], AF.Ln, bias=1.0)

            for pt in range(NPT):
                nc.vector.tensor_mul(uT[:, pt, :], dtT[:, pt, :],
                                     xc_own[:, pt, :].bitcast(F32))

        # ---------- Phase C: selective scan ----------
        with tc.tile_pool(name="pC", bufs=2) as pC, \
             tc.tile_pool(name="psC", bufs=8, space="PSUM") as psC:
            for pair in range(2):
                pts = (2 * pair, 2 * pair + 1)
                # y accumulators: one PSUM bank per (pt-in-pair, token chunk)
                yps = {}
                for ptl, pt in enumerate(pts):
                    for q in range(NC_):
                        yps_t = psC.tile([128, 512], F32, tag="yps")
                        yps[(ptl, q)] = yps_t
                        # initialize with D * xc via diag matmul
                        nc.tensor.matmul(
                            yps_t[:], sb_dD[:, pt, :],
                            xc_own[:, pt, q * 512:(q + 1) * 512],
                            start=True, stop=False, skip_group_check=True)
                for s in range(D_STATE):
                    B_bc = pC.tile([128, SEQ], BF16, tag="bbc")
                    C_bc = pC.tile([128, SEQ], BF16, tag="cbc")
                    brow = bcd[s:s + 1, :]
                    crow = bcd[D_STATE + s:D_STATE + s + 1, :]
                    nc.gpsimd.dma_start(B_bc[:], bass.AP(
                        tensor=brow.tensor, offset=brow.offset,
                        ap=[[0, 128]] + list(brow.ap[1:])))
                    nc.gpsimd.dma_start(C_bc[:], bass.AP(
                        tensor=crow.tensor, offset=crow.offset,
                        ap=[[0, 128]] + list(crow.ap[1:])))
                    # high-s multiplies run on GpSimd to offload VectorE
                    # (the scan opcode itself is VectorE-only)
                    gps_thresh = int(os.environ.get("BIMAMBA_GPS", "16"))
                    eng = nc.gpsimd if s >= gps_thresh else nc.vector
                    for ptl, pt in enumerate(pts):
                        dA = pC.tile([128, SEQ], F32R, tag="dA")
                        nc.scalar.activation(dA[:], dtT[:, pt, :], AF.Exp,
                                             scale=sb_A[:, pt, s:s + 1])
                        dBu = pC.tile([128, SEQ], BF16, tag="dBu")
                        eng.tensor_mul(dBu[:], uT[:, pt, :], B_bc[:])
                        h = pC.tile([128, SEQ], BF16, tag="h")
                        nc.vector.tensor_tensor_scan(h[:], dA[:], dBu[:], 0.0,
                                                     OP.mult, OP.add)
                        hc = pC.tile([128, SEQ], BF16, tag="hc")
                        eng.tensor_mul(hc[:], h[:], C_bc[:])
                        for q in range(NC_):
                            nc.tensor.matmul(
                                yps[(ptl, q)][:], sb_id[:],
                                hc[:, q * 512:(q + 1) * 512],
                                start=False, stop=(s == D_STATE - 1),
                                skip_group_check=True)
                # gate: y_g = y * silu(z)
                for ptl, pt in enumerate(pts):
                    for q in range(NC_):
                        nc.vector.tensor_mul(
                            y_g[:, pt, q * 512:(q + 1) * 512],
                            yps[(ptl, q)][:],
                            gT[:, pt, q * 512:(q + 1) * 512])

        # ---------- Phase D: out_proj ----------
        with tc.tile_pool(name="pD", bufs=1) as pD, \
             tc.tile_pool(name="pDo", bufs=3) as pDo, \
             tc.tile_pool(name="psD", bufs=3, space="PSUM") as psD:
            sb_wout = pD.tile([128, NPT, DIM], F32R)
            nc.sync.dma_start(sb_wout[:], w_out[:])
            for mt in range(NPT):
                for c in range(NC_):
                    ps4 = psD.tile([128, 512], F32, tag="mm")
                    for k in range(NPT):
                        nc.tensor.matmul(
                            ps4[:], sb_wout[:, k, mt * 128:(mt + 1) * 128],
                            y_g[:, k, c * 512:(c + 1) * 512],
                            start=(k == 0), stop=(k == NPT - 1))
                    ot = pDo.tile([128, 512], F32, tag="ot")
                    nc.scalar.activation(ot[:], ps4[:], AF.Copy)
                    nc.sync.dma_start(oT[:, mt, c * 512:(c + 1) * 512], ot[:])

    nc.compile()
    _PROG_CACHE["nc"] = nc
    return nc


def _prep_core_inputs(x, params, direction, batch, half):
    in_w, conv_w, conv_b, xproj_w, dt_w, dt_b, A_log, D, out_w = params
    xb = x[batch]
    if direction == 1:
        xb = xb[::-1]
    xT = np.ascontiguousarray(xb.T)

    own = np.arange(half * HALF, (half + 1) * HALF)
    other = np.arange((1 - half) * HALF, (2 - half) * HALF)
    perm = np.concatenate([own, other])

    w_in = np.concatenate([in_w[perm], in_w[D_INNER + own]], axis=0).T  # [512, 1536]
    cw = conv_w[perm, 0, :]                                            # [1024, 4]
    convdiag = np.zeros((128, D_CONV, NFT, 128), np.float32)
    ii = np.arange(128)
    for k in range(D_CONV):
        for m in range(NFT):
            convdiag[ii, k, m, ii] = cw[m * 128:(m + 1) * 128, k]
    convb_ = conv_b[perm].reshape(NFT, 128, 1).transpose(1, 0, 2)
    w_xp = xproj_w[:, perm].T.reshape(NFT, 128, -1).transpose(1, 0, 2)
    w_dt_ = np.ascontiguousarray(dt_w[own].T)
    dtb_ = dt_b[own].reshape(NPT, 128, 1).transpose(1, 0, 2)
    Acol_ = (-np.exp(A_log[own])).reshape(NPT, 128, D_STATE).transpose(1, 0, 2)
    dD = np.zeros((128, NPT, 128), np.float32)
    Dr = D[own].reshape(NPT, 128)
    for ptn in range(NPT):
        dD[ii, ptn, ii] = Dr[ptn]
    w_out_ = out_w[:, own].T.reshape(NPT, 128, DIM).transpose(1, 0, 2)

    def c32(a):
        return np.ascontiguousarray(a, dtype=np.float32)

    return {
        "xT": c32(xT.reshape(4, 128, SEQ).transpose(1, 0, 2)),
        "w_in": c32(w_in.reshape(4, 128, -1).transpose(1, 0, 2)),
        "convdiag": c32(convdiag),
        "convb": c32(convb_),
        "w_xp": c32(w_xp),
        "w_dt": c32(w_dt_),
        "dtb": c32(dtb_),
        "Acol": c32(Acol_),
        "diagD": c32(dD),
        "ident": np.eye(128, dtype=NPBF16),
        "w_out": c32(w_out_),
        "zero3": np.zeros((128, 3), np.float32),
    }


def _run(nc, in_maps):
    if os.environ.get("BIMAMBA_SIM"):
        from concourse.bass_interp import CoreSim
        results = []
        n = int(os.environ.get("BIMAMBA_SIM_CORES", "8"))
        for m in in_maps[:n]:
            sim = CoreSim(nc)
            for k, v in m.items():
                sim.tensor(k)[:] = v
            sim.simulate()
            results.append({"oT": np.array(sim.tensor("oT"))})
        return results
    return run_bass_kernel_spmd(nc, in_maps, core_ids=list(range(8))).results


def _prep_all(inputs):
    x = np.asarray(inputs["x"], np.float32)
    names = ["in_w", "conv_w", "conv_b", "xproj_w", "dt_w", "dt_b", "A_log", "D", "out_w"]
    fp = tuple(np.asarray(inputs["f_" + n], np.float32) for n in names)
    rp = tuple(np.asarray(inputs["r_" + n], np.float32) for n in names)
    in_maps = []
    meta = []
    for d in (0, 1):
        for b in range(B_SZ):
            for h in (0, 1):
                in_maps.append(_prep_core_inputs(x, fp if d == 0 else rp, d, b, h))
                meta.append((d, b, h))
    return in_maps, meta


def kernel(**inputs):
    nc = _build_program()
    in_maps, meta = _prep_all(inputs)
    results = _run(nc, in_maps)

    acc = np.zeros((2, B_SZ, SEQ, DIM), np.float32)
    for (d, b, h), r in zip(meta, results):
        oTv = r["oT"]
        o = oTv.transpose(1, 0, 2).reshape(DIM, SEQ).T
        if d == 1:
            o = o[::-1]
        acc[d, b] += o
    out = 0.5 * (acc[0] + acc[1])
    return out.astype(np.float32)
